# revision 23
# baseline (speedup 1.0000x reference)
"""GQA causal-attention prefill kernel for 8 Trainium2 NeuronCores.

Reference computation (B=2, S=2048, D=4096, Q=32 q-heads, N=8 kv-heads,
H=128): QKV projection + RoPE + causal GQA attention + O projection.

Sharding: core c handles batch b = c//4 and kv-head pair g = c%4
(kv-heads 2g..2g+1, q-heads 8g..8g+7).  No collectives: each core
computes its partial o-projection (sum over its 8 q-heads) and the host
sums the four partials per batch at gather time (the "all-reduce").

Device-side layout strategy (per core):
  - x is fed pre-transposed ([D, S]) so projections contract over D with
    matmuls (stationary = weight / xT tile, moving N = s-chunk).
  - q, k are produced in [h, s] layout; v in [t, h] layout.
  - scores are computed transposed (S^T = K^T Q, psum [t, s]) so the
    softmax weights feed the AV matmul directly as the moving operand
    with t as the contraction partition - no transposes anywhere.
  - softmax denominators via ones-vector matmuls accumulated alongside
    AV; 1/den broadcast across partitions via a K=1 outer-product
    matmul; exp on ScalarE (fused 1/sqrt(H) scaling); causal masking via
    a triangular additive tile on the block diagonal.
  - RoPE in [h, s] layout: sign-folded sin table, halves swapped by a
    64-partition SBUF->SBUF DMA, then one add.
  - all matmuls run in float32r (fp32 bits, reduced-precision fast path).
"""

import math
import sys

import numpy as np

for _p in ("/opt/trn_rl_repo", "/root/.axon_site/_ro/trn_rl_repo"):
    if _p not in sys.path:
        sys.path.append(_p)

import concourse.bacc as bacc
import concourse.mybir as mybir
import concourse.tile as tile
from concourse import bass_utils

dt = mybir.dt
F32 = dt.float32
F32R = dt.float32r
ADD = mybir.AluOpType.add
MULT = mybir.AluOpType.mult
EXP = mybir.ActivationFunctionType.Exp

# Full-problem config (per core after sharding).
FULL_CFG = dict(S=2048, D=4096, QH=8, KH=2, H=128, SC=256, ST=512, HG=4)
N_CORES = 8
ROPE_THETA = 10000.0
NEG_BIG = -1.0e30


def build_bass(cfg):
    S, D, QH, KH, H = cfg["S"], cfg["D"], cfg["QH"], cfg["KH"], cfg["H"]
    SC, ST, HG = cfg["SC"], cfg["ST"], cfg["HG"]
    assert H == 128 and D % 128 == 0 and S % SC == 0 and SC % 128 == 0
    assert S % ST == 0 and ST % 128 == 0 and QH % KH == 0 and QH % HG == 0
    DT = D // 128          # d-tiles (contraction tiles for projections)
    NCH = S // SC          # s-chunks for projections
    NJ = S // ST           # s-tiles for attention
    TJ = ST // 128         # 128-wide t-tiles per attention s-tile
    NT = S // 128          # total t-tiles
    G = QH // KH           # GQA group size
    EW = 512               # o-proj output tile width
    NE = D // EW
    scale = 1.0 / math.sqrt(H)

    from contextlib import ExitStack

    nc = bacc.Bacc("TRN2", target_bir_lowering=False, debug=False,
                   enable_asserts=False, num_devices=N_CORES)

    xT = nc.dram_tensor("xT", [D, S], F32R, kind="ExternalInput")
    wq = nc.dram_tensor("wq", [QH, D, H], F32R, kind="ExternalInput")
    wk = nc.dram_tensor("wk", [KH, D, H], F32R, kind="ExternalInput")
    wv = nc.dram_tensor("wv", [KH, D, H], F32R, kind="ExternalInput")
    wo = nc.dram_tensor("wo", [QH, H, D], F32R, kind="ExternalInput")
    cos_d = nc.dram_tensor("cos_t", [128, S], F32, kind="ExternalInput")
    sin_d = nc.dram_tensor("sin_t", [128, S], F32, kind="ExternalInput")
    tri_d = nc.dram_tensor("tri_t", [128, 128], F32, kind="ExternalInput")
    ones_d = nc.dram_tensor("ones_t", [128, 128], F32R, kind="ExternalInput")
    perm_d = nc.dram_tensor("perm_t", [128, 128], F32R, kind="ExternalInput")
    o_out = nc.dram_tensor("o_out", [S, D], F32, kind="ExternalOutput")

    xT_r = xT.ap().rearrange("(dt p) s -> p dt s", p=128)

    with tile.TileContext(nc) as tc, \
         nc.allow_low_precision(reason="deliberate fp32r matmul pipeline"):
        with tc.tile_pool(name="persist", bufs=1) as persist, \
             tc.tile_pool(name="drsc", bufs=1, space="DRAM") as dram:
            cos_sb = persist.tile([128, S], F32)
            sin_sb = persist.tile([128, S], F32)
            tri_sb = persist.tile([128, 128], F32)
            ones_sb = persist.tile([128, 128], F32R)
            perm_sb = persist.tile([128, 128], F32R)
            k_sb = persist.tile([128, KH, S], F32R)
            v_sb = persist.tile([128, NT, KH * H], F32R)
            ones_col = ones_sb[:, 0:1]
            ones_row = ones_sb[0:1, :]

            def load_tables():
                nc.sync.dma_start(cos_sb[:], cos_d[:, :])
                nc.sync.dma_start(sin_sb[:], sin_d[:, :])
                nc.sync.dma_start(tri_sb[:], tri_d[:, :])
                nc.sync.dma_start(ones_sb[:], ones_d[:, :])
                nc.sync.dma_start(perm_sb[:], perm_d[:, :])

            oda = [dram.tile([128, S], F32R, tag="oda%d" % h,
                             name="oda%d" % h)
                   for h in range(QH)]
            den_d = [dram.tile([1, S], F32, tag="den%d" % h,
                               name="den%d" % h)
                     for h in range(QH)]
            den_d = [dram.tile([1, S], F32, tag="den%d" % h,
                               name="den%d" % h)
                     for h in range(QH)]

            def rope(ps_tile, dst_ap, s0, W, rp, swp, swtag):
                """dst = rope(ps_tile) for s-range [s0, s0+W).

                The rotate-half partition swap runs on TensorE as a
                matmul with a constant permutation matrix (sign folded
                into the sin table), keeping the chain DMA-free."""
                ta = rp.tile([128, W], F32, tag="ta")
                tb = rp.tile([128, W], F32R, tag="tb")
                csl = cos_sb[:, s0:s0 + W]
                ssl = sin_sb[:, s0:s0 + W]
                nc.vector.tensor_tensor(ta[:], ps_tile, csl, MULT)
                nc.vector.tensor_tensor(tb[:], ps_tile, ssl, MULT)
                tbs = swp.tile([128, W], F32, tag=swtag)
                nc.tensor.matmul(tbs[:], perm_sb[:], tb[:],
                                 start=True, stop=True)
                nc.vector.tensor_tensor(dst_ap, ta[:], tbs[:], ADD)

            # wq head tiles live in their own pool opened before pass A
            # so the first q-weight load overlaps k/v projection.
            wqp_es = ExitStack()
            wqp0 = wqp_es.enter_context(tc.tile_pool(name="wqp0", bufs=1))
            wq_pre = wqp0.tile([128, DT, H], F32R, tag="wq0", name="wq_pre")
            xtp_es = ExitStack()
            xtsp = xtp_es.enter_context(tc.tile_pool(name="xts", bufs=2))
            xtp_es = ExitStack()
            xtsp = xtp_es.enter_context(tc.tile_pool(name="xts", bufs=2))

            # ---- PASS A: k and v projections (+ RoPE on k) ----
            with nc.named_scope("passA"), \
                 tc.tile_pool(name="wkv", bufs=1) as wkvp, \
                 tc.tile_pool(name="ropeA", bufs=2) as rpA, \
                 tc.tile_pool(name="pskA", bufs=3, space="PSUM") as psk, \
                 tc.tile_pool(name="psvA", bufs=3, space="PSUM") as psv, \
                 tc.tile_pool(name="pswA", bufs=2, space="PSUM") as psw:
                wk_t = wkvp.tile([128, KH, DT, H], F32R)
                wv_t = wkvp.tile([128, DT, KH, H], F32R)
                first_loads_done = False
                for ch in range(NCH):
                    xts = xtsp.tile([128, DT, SC], F32R, tag="xts")
                    hdt = DT // 2
                    nc.sync.dma_start(xts[:, 0:hdt],
                                      xT_r[:, 0:hdt, ch * SC:(ch + 1) * SC])
                    nc.sync.dma_start(xts[:, hdt:DT],
                                      xT_r[:, hdt:DT, ch * SC:(ch + 1) * SC])
                    if not first_loads_done:
                        first_loads_done = True
                        hd = DT // 2
                        for half_ in range(2):
                            dsl = slice(half_ * hd, (half_ + 1) * hd)
                            for n in range(KH):
                                nc.sync.dma_start(
                                    wk_t[:, n, dsl],
                                    wk.ap()[n].rearrange(
                                        "(dt p) h -> p dt h", p=128)[:, dsl])
                            for n in range(KH):
                                nc.sync.dma_start(
                                    wv_t[:, dsl, n, :],
                                    wv.ap()[n].rearrange(
                                        "(dt p) h -> p dt h", p=128)[:, dsl])
                        load_tables()
                        nc.sync.dma_start(
                            wq_pre[:],
                            wq.ap()[0].rearrange("(dt p) h -> p dt h", p=128))
                    for kh in range(KH):
                        pk = psk.tile([128, SC], F32, tag="pk")
                        for di in range(DT):
                            nc.tensor.matmul(
                                pk[:],
                                wk_t[:, kh, di, :],
                                xts[:, di, :],
                                start=(di == 0), stop=(di == DT - 1))
                        rope(pk[:], k_sb[:, kh, ch * SC:(ch + 1) * SC], ch * SC,
                             SC, rpA, psw, "tbs")
                    for tl in range(SC // 128):
                        pv = psv.tile([128, KH * H], F32, tag="pv")
                        for di in range(DT):
                            nc.tensor.matmul(
                                pv[:],
                                xts[:, di, tl * 128:(tl + 1) * 128],
                                wv_t[:, di].rearrange(
                                    "p a b -> p (a b)"),
                                start=(di == 0), stop=(di == DT - 1))
                        tt = ch * (SC // 128) + tl
                        nc.vector.tensor_copy(v_sb[:, tt, :], pv[:])

            # ---- FUSED PASS: q projection + RoPE + attention ----
            # Per (head-group, s-tile): project q for HG heads straight
            # into SBUF, then run their causal attention.  Projection
            # matmuls of iteration i+1 overlap attention of iteration i.
            es = ExitStack()
            with es:
                es.enter_context(wqp_es.pop_all())
                es.enter_context(xtp_es.pop_all())
                wqp = es.enter_context(tc.tile_pool(name="wqp", bufs=1))
                rpB = es.enter_context(tc.tile_pool(name="ropeB", bufs=2))
                qjp = es.enter_context(tc.tile_pool(name="qj", bufs=5))
                wtp = es.enter_context(tc.tile_pool(name="wt", bufs=4))
                rcpp = es.enter_context(tc.tile_pool(name="rcp", bufs=2))
                onp = es.enter_context(tc.tile_pool(name="on", bufs=2))
                psq = es.enter_context(
                    tc.tile_pool(name="psqB", bufs=2, space="PSUM"))
                pss = es.enter_context(
                    tc.tile_pool(name="pss", bufs=3, space="PSUM"))
                pso = es.enter_context(
                    tc.tile_pool(name="pso", bufs=2, space="PSUM"))
                psd = es.enter_context(
                    tc.tile_pool(name="psd", bufs=1, space="PSUM"))

                def qproj_half(wq_h, qj, j, half):
                    xts = xtsp.tile([128, DT, SC], F32R, tag="xts",
                                    name="xtsF%d" % id(qj))
                    s0 = j * ST + half * SC
                    nc.sync.dma_start(xts[:], xT_r[:, :, s0:s0 + SC])
                    for hl in range(HG):
                        pq = psq.tile([128, SC], F32, tag="pq")
                        for di in range(DT):
                            nc.tensor.matmul(
                                pq[:], wq_h[hl][:, di, :], xts[:, di, :],
                                start=(di == 0), stop=(di == DT - 1))
                        rope(pq[:], qj[hl][:, half * SC:(half + 1) * SC],
                             s0, SC, rpB, pss, "ps")

                def attn_head(h, qjt, j):
                    kh = h // G
                    po = pso.tile([128, ST], F32, tag="po")
                    pden = psd.tile([1, ST], F32, tag="pden")
                    KT = (j + 1) * TJ
                    for kt in range(KT):
                        ps = pss.tile([128, ST], F32, tag="ps")
                        nc.tensor.matmul(
                            ps[:], k_sb[:, kh, kt * 128:(kt + 1) * 128],
                            qjt[:], start=True, stop=True)
                        wtile = wtp.tile([128, ST], F32R, tag="wt")
                        m = kt - j * TJ
                        lo = 0
                        if m >= 0:
                            nc.vector.tensor_tensor(
                                ps[:, m * 128:(m + 1) * 128],
                                ps[:, m * 128:(m + 1) * 128],
                                tri_sb[:], ADD)
                            lo = m * 128
                            if m > 0:
                                nc.scalar.mul(wtile[:, 0:lo], ps[:, 0:lo], 0.0)
                        nc.scalar.activation(wtile[:, lo:ST], ps[:, lo:ST],
                                             EXP, scale=scale)
                        nc.tensor.matmul(
                            po[:], v_sb[:, kt, kh * H:(kh + 1) * H], wtile[:],
                            start=(kt == 0), stop=(kt == KT - 1))
                        nc.tensor.matmul(
                            pden[:], ones_col, wtile[:],
                            start=(kt == 0), stop=(kt == KT - 1))
                    rcp = rcpp.tile([1, ST], F32, tag="rcp")
                    nc.vector.reciprocal(rcp[:], pden[:])
                    nc.scalar.dma_start(den_d[h][:, j * ST:(j + 1) * ST],
                                        rcp[:])
                    on = onp.tile([128, ST], F32R, tag="on")
                    nc.vector.tensor_copy(on[:], po[:])
                    nc.scalar.dma_start(oda[h][:, j * ST:(j + 1) * ST],
                                        on[:])

                for hg in range(QH // HG):
                    wq_h = []
                    for hl in range(HG):
                        if hg == 0 and hl == 0:
                            wq_h.append(wq_pre)
                            continue
                        pool_ = wqp0 if hl == 0 else wqp
                        wt_ = pool_.tile([128, DT, H], F32R, tag="wq%d" % hl,
                                         name="wq_%d_%d" % (hg, hl))
                        nc.sync.dma_start(
                            wt_[:],
                            wq.ap()[hg * HG + hl].rearrange(
                                "(dt p) h -> p dt h", p=128))
                        wq_h.append(wt_)
                    for j in range(NJ):
                        qj = [qjp.tile([128, ST], F32R, tag="qj",
                                       name="qj%d_%d_%d" % (hg, j, hl))
                              for hl in range(HG)]
                        for half in range(ST // SC):
                            qproj_half(wq_h, qj, j, half)
                        for hl in range(HG):
                            attn_head(hg * HG + hl, qj[hl], j)

            # ---- PASS C: o projection (partial over this core's heads) ----
            with nc.named_scope("passC"), \
                 tc.tile_pool(name="otp", bufs=1) as otp, \
                 tc.tile_pool(name="wop", bufs=2) as wop, \
                 tc.tile_pool(name="ocp", bufs=3) as ocp, \
                 tc.tile_pool(name="psc", bufs=4, space="PSUM") as psc:
                ot = otp.tile([128, QH, S], F32R)
                for h in range(QH):
                    nc.sync.dma_start(ot[:, h, :], oda[h][:, :])
                with tc.tile_pool(name="bch", bufs=2) as bchp:
                    for h in range(QH):
                        bch = bchp.tile([128, S], F32, tag="bch",
                                        name="bch%d" % h)
                        nc.sync.dma_start(bch[:],
                                          den_d[h][:].partition_broadcast(128))
                        nc.vector.tensor_tensor(ot[:, h, :], ot[:, h, :],
                                                bch[:], MULT)
                with tc.tile_pool(name="bch", bufs=2) as bchp:
                    for h in range(QH):
                        bch = bchp.tile([128, S], F32, tag="bch",
                                        name="bch%d" % h)
                        nc.sync.dma_start(bch[:],
                                          den_d[h][:].partition_broadcast(128))
                        nc.vector.tensor_tensor(ot[:, h, :], ot[:, h, :],
                                                bch[:], MULT)
                for e in range(NE):
                    woe = wop.tile([128, QH, EW], F32R, tag="woe")
                    for h in range(QH):
                        nc.sync.dma_start(
                            woe[:, h, :], wo.ap()[h, :, e * EW:(e + 1) * EW])
                    for st in range(S // 128):
                        pc = psc.tile([128, EW], F32, tag="pc")
                        for h in range(QH):
                            nc.tensor.matmul(
                                pc[:],
                                ot[:, h, st * 128:(st + 1) * 128],
                                woe[:, h, :],
                                start=(h == 0), stop=(h == QH - 1))
                        oc = ocp.tile([128, EW], F32, tag="oc")
                        nc.vector.tensor_copy(oc[:], pc[:])
                        nc.scalar.dma_start(
                            o_out[st * 128:(st + 1) * 128, e * EW:(e + 1) * EW],
                            oc[:])

    nc.compile()
    return nc


def _perm_matrix():
    P = np.zeros((128, 128), dtype=np.float32)
    P[np.arange(128), (np.arange(128) + 64) % 128] = 1.0
    return P


def make_tables(positions_b, S, H):
    """cos/sin tables in [128, S] layout with the sign fold for the swap
    trick (rows 0:63 -> +sin, 64:127 -> -sin), plus the triangular mask."""
    half = H // 2
    inv_freq = 1.0 / (ROPE_THETA ** (np.arange(half, dtype=np.float64) * 2.0 / H))
    ang = positions_b.astype(np.float64)[None, :] * inv_freq[:, None]  # [half, S]
    cos_h = np.cos(ang)
    sin_h = np.sin(ang)
    cos_t = np.concatenate([cos_h, cos_h], axis=0).astype(np.float32)
    sin_t = np.concatenate([sin_h, -sin_h], axis=0).astype(np.float32)
    idx = np.arange(128)
    tri = np.where(idx[:, None] <= idx[None, :], 0.0, NEG_BIG).astype(np.float32)
    return cos_t, sin_t, tri


def make_in_maps(x, positions, Wq, Wk, Wv, Wo, cfg):
    """Shard the full inputs into the 8 per-core input maps."""
    QH, KH = cfg["QH"], cfg["KH"]
    S, H = cfg["S"], cfg["H"]
    B = x.shape[0]
    groups = N_CORES // B
    tables = [make_tables(np.asarray(positions[b]), S, H) for b in range(B)]
    in_maps = []
    for c in range(N_CORES):
        b, g = divmod(c, groups)
        cos_t, sin_t, tri = tables[b]
        in_maps.append({
            "xT": np.ascontiguousarray(np.asarray(x[b]).T),
            "wq": np.ascontiguousarray(Wq[g * QH:(g + 1) * QH]),
            "wk": np.ascontiguousarray(Wk[g * KH:(g + 1) * KH]),
            "wv": np.ascontiguousarray(Wv[g * KH:(g + 1) * KH]),
            "wo": np.ascontiguousarray(Wo[g * QH:(g + 1) * QH]),
            "cos_t": cos_t,
            "sin_t": sin_t,
            "tri_t": tri,
            "ones_t": np.ones((128, 128), dtype=np.float32),
            "perm_t": _perm_matrix(),
        })
    return in_maps


_NC_CACHE = {}


def _get_nc(cfg_key=None):
    cfg = FULL_CFG if cfg_key is None else cfg_key
    key = tuple(sorted(cfg.items()))
    if key not in _NC_CACHE:
        _NC_CACHE[key] = build_bass(cfg)
    return _NC_CACHE[key]


def run(x, positions, Wq, Wk, Wv, Wo, trace=False, trace_kwargs=None):
    cfg = FULL_CFG
    nc = _get_nc(cfg)
    in_maps = make_in_maps(np.asarray(x), np.asarray(positions),
                           np.asarray(Wq), np.asarray(Wk), np.asarray(Wv),
                           np.asarray(Wo), cfg)
    res = bass_utils.run_bass_kernel_spmd(
        nc, in_maps, list(range(N_CORES)), trace=trace,
        **(trace_kwargs or {}))
    B = np.asarray(x).shape[0]
    groups = N_CORES // B
    outs = []
    for b in range(B):
        acc = res.results[b * groups]["o_out"].astype(np.float64)
        for g in range(1, groups):
            acc += res.results[b * groups + g]["o_out"]
        outs.append(acc.astype(np.float32))
    return np.stack(outs, axis=0), res


def kernel(x, positions, Wq, Wk, Wv, Wo):
    out, _ = run(x, positions, Wq, Wk, Wv, Wo, trace=False)
    return out


# revision 24
# speedup vs baseline: 1.0309x; 1.0309x over previous
"""GQA causal-attention prefill kernel for 8 Trainium2 NeuronCores.

Reference computation (B=2, S=2048, D=4096, Q=32 q-heads, N=8 kv-heads,
H=128): QKV projection + RoPE + causal GQA attention + O projection.

Sharding: core c handles batch b = c//4 and kv-head pair g = c%4
(kv-heads 2g..2g+1, q-heads 8g..8g+7).  No collectives: each core
computes its partial o-projection (sum over its 8 q-heads) and the host
sums the four partials per batch at gather time (the "all-reduce").

Device-side layout strategy (per core):
  - x is fed pre-transposed ([D, S]) so projections contract over D with
    matmuls (stationary = weight / xT tile, moving N = s-chunk).
  - q, k are produced in [h, s] layout; v in [t, h] layout.
  - scores are computed transposed (S^T = K^T Q, psum [t, s]) so the
    softmax weights feed the AV matmul directly as the moving operand
    with t as the contraction partition - no transposes anywhere.
  - softmax denominators via ones-vector matmuls accumulated alongside
    AV; 1/den broadcast across partitions via a K=1 outer-product
    matmul; exp on ScalarE (fused 1/sqrt(H) scaling); causal masking via
    a triangular additive tile on the block diagonal.
  - RoPE in [h, s] layout: sign-folded sin table, halves swapped by a
    64-partition SBUF->SBUF DMA, then one add.
  - all matmuls run in float32r (fp32 bits, reduced-precision fast path).
"""

import math
import sys

import numpy as np

for _p in ("/opt/trn_rl_repo", "/root/.axon_site/_ro/trn_rl_repo"):
    if _p not in sys.path:
        sys.path.append(_p)

import concourse.bacc as bacc
import concourse.mybir as mybir
import concourse.tile as tile
from concourse import bass_utils

dt = mybir.dt
F32 = dt.float32
F32R = dt.float32r
ADD = mybir.AluOpType.add
MULT = mybir.AluOpType.mult
EXP = mybir.ActivationFunctionType.Exp

# Full-problem config (per core after sharding).
FULL_CFG = dict(S=2048, D=4096, QH=8, KH=2, H=128, SC=256, ST=512, HG=4)
N_CORES = 8
ROPE_THETA = 10000.0
NEG_BIG = -1.0e30


def build_bass(cfg):
    S, D, QH, KH, H = cfg["S"], cfg["D"], cfg["QH"], cfg["KH"], cfg["H"]
    SC, ST, HG = cfg["SC"], cfg["ST"], cfg["HG"]
    assert H == 128 and D % 128 == 0 and S % SC == 0 and SC % 128 == 0
    assert S % ST == 0 and ST % 128 == 0 and QH % KH == 0 and QH % HG == 0
    DT = D // 128          # d-tiles (contraction tiles for projections)
    NCH = S // SC          # s-chunks for projections
    NJ = S // ST           # s-tiles for attention
    TJ = ST // 128         # 128-wide t-tiles per attention s-tile
    NT = S // 128          # total t-tiles
    G = QH // KH           # GQA group size
    EW = 512               # o-proj output tile width
    NE = D // EW
    scale = 1.0 / math.sqrt(H)

    from contextlib import ExitStack

    nc = bacc.Bacc("TRN2", target_bir_lowering=False, debug=False,
                   enable_asserts=False, num_devices=N_CORES)

    xT = nc.dram_tensor("xT", [D, S], F32R, kind="ExternalInput")
    wq = nc.dram_tensor("wq", [QH, D, H], F32R, kind="ExternalInput")
    wk = nc.dram_tensor("wk", [KH, D, H], F32R, kind="ExternalInput")
    wv = nc.dram_tensor("wv", [KH, D, H], F32R, kind="ExternalInput")
    wo = nc.dram_tensor("wo", [QH, H, D], F32R, kind="ExternalInput")
    cos_d = nc.dram_tensor("cos_t", [128, S], F32, kind="ExternalInput")
    sin_d = nc.dram_tensor("sin_t", [128, S], F32, kind="ExternalInput")
    tri_d = nc.dram_tensor("tri_t", [128, 128], F32, kind="ExternalInput")
    ones_d = nc.dram_tensor("ones_t", [128, 128], F32R, kind="ExternalInput")
    perm_d = nc.dram_tensor("perm_t", [128, 128], F32R, kind="ExternalInput")
    o_out = nc.dram_tensor("o_out", [S, D], F32, kind="ExternalOutput")

    xT_r = xT.ap().rearrange("(dt p) s -> p dt s", p=128)

    with tile.TileContext(nc) as tc, \
         nc.allow_low_precision(reason="deliberate fp32r matmul pipeline"):
        with tc.tile_pool(name="persist", bufs=1) as persist, \
             tc.tile_pool(name="drsc", bufs=1, space="DRAM") as dram:
            cos_sb = persist.tile([128, S], F32)
            sin_sb = persist.tile([128, S], F32)
            tri_sb = persist.tile([128, 128], F32)
            ones_sb = persist.tile([128, 128], F32R)
            perm_sb = persist.tile([128, 128], F32R)
            k_sb = persist.tile([128, KH, S], F32R)
            v_sb = persist.tile([128, NT, KH * H], F32R)
            ones_col = ones_sb[:, 0:1]
            ones_row = ones_sb[0:1, :]

            def load_tables():
                nc.sync.dma_start(cos_sb[:], cos_d[:, :])
                nc.sync.dma_start(sin_sb[:], sin_d[:, :])
                nc.sync.dma_start(tri_sb[:], tri_d[:, :])
                nc.sync.dma_start(ones_sb[:], ones_d[:, :])
                nc.sync.dma_start(perm_sb[:], perm_d[:, :])

            oda = [dram.tile([128, S], F32R, tag="oda%d" % h,
                             name="oda%d" % h)
                   for h in range(QH)]
            den_d = [dram.tile([1, S], F32, tag="den%d" % h,
                               name="den%d" % h)
                     for h in range(QH)]
            den_d = [dram.tile([1, S], F32, tag="den%d" % h,
                               name="den%d" % h)
                     for h in range(QH)]

            def rope(ps_tile, dst_ap, s0, W, rp, swp, swtag):
                """dst = rope(ps_tile) for s-range [s0, s0+W).

                The rotate-half partition swap runs on TensorE as a
                matmul with a constant permutation matrix (sign folded
                into the sin table), keeping the chain DMA-free."""
                ta = rp.tile([128, W], F32, tag="ta")
                tb = rp.tile([128, W], F32R, tag="tb")
                csl = cos_sb[:, s0:s0 + W]
                ssl = sin_sb[:, s0:s0 + W]
                nc.vector.tensor_tensor(ta[:], ps_tile, csl, MULT)
                nc.vector.tensor_tensor(tb[:], ps_tile, ssl, MULT)
                tbs = swp.tile([128, W], F32, tag=swtag)
                nc.tensor.matmul(tbs[:], perm_sb[:], tb[:],
                                 start=True, stop=True)
                nc.vector.tensor_tensor(dst_ap, ta[:], tbs[:], ADD)

            # wq head tiles live in their own pool opened before pass A
            # so the first q-weight load overlaps k/v projection.
            wqp_es = ExitStack()
            wqp0 = wqp_es.enter_context(tc.tile_pool(name="wqp0", bufs=1))
            wq_pre = wqp0.tile([128, DT, H], F32R, tag="wq0", name="wq_pre")
            xtp_es = ExitStack()
            xtsp = xtp_es.enter_context(tc.tile_pool(name="xts", bufs=2))
            xtp_es = ExitStack()
            xtsp = xtp_es.enter_context(tc.tile_pool(name="xts", bufs=2))

            # ---- PASS A: k and v projections (+ RoPE on k) ----
            with nc.named_scope("passA"), \
                 tc.tile_pool(name="wkv", bufs=1) as wkvp, \
                 tc.tile_pool(name="ropeA", bufs=2) as rpA, \
                 tc.tile_pool(name="pskA", bufs=3, space="PSUM") as psk, \
                 tc.tile_pool(name="psvA", bufs=3, space="PSUM") as psv, \
                 tc.tile_pool(name="pswA", bufs=2, space="PSUM") as psw:
                wk_t = wkvp.tile([128, KH, DT, H], F32R)
                wv_t = wkvp.tile([128, DT, KH, H], F32R)
                first_loads_done = False
                for ch in range(NCH):
                    xts = xtsp.tile([128, DT, SC], F32R, tag="xts")
                    hdt = DT // 2
                    nc.sync.dma_start(xts[:, 0:hdt],
                                      xT_r[:, 0:hdt, ch * SC:(ch + 1) * SC])
                    nc.sync.dma_start(xts[:, hdt:DT],
                                      xT_r[:, hdt:DT, ch * SC:(ch + 1) * SC])
                    if not first_loads_done:
                        first_loads_done = True
                        hd = DT // 2
                        for half_ in range(2):
                            dsl = slice(half_ * hd, (half_ + 1) * hd)
                            for n in range(KH):
                                nc.sync.dma_start(
                                    wk_t[:, n, dsl],
                                    wk.ap()[n].rearrange(
                                        "(dt p) h -> p dt h", p=128)[:, dsl])
                            for n in range(KH):
                                nc.sync.dma_start(
                                    wv_t[:, dsl, n, :],
                                    wv.ap()[n].rearrange(
                                        "(dt p) h -> p dt h", p=128)[:, dsl])
                        load_tables()
                        nc.sync.dma_start(
                            wq_pre[:],
                            wq.ap()[0].rearrange("(dt p) h -> p dt h", p=128))
                    for kh in range(KH):
                        pk = psk.tile([128, SC], F32, tag="pk")
                        for di in range(DT):
                            nc.tensor.matmul(
                                pk[:],
                                wk_t[:, kh, di, :],
                                xts[:, di, :],
                                start=(di == 0), stop=(di == DT - 1))
                        rope(pk[:], k_sb[:, kh, ch * SC:(ch + 1) * SC], ch * SC,
                             SC, rpA, psw, "tbs")
                    for tl in range(SC // 128):
                        pv = psv.tile([128, KH * H], F32, tag="pv")
                        for di in range(DT):
                            nc.tensor.matmul(
                                pv[:],
                                xts[:, di, tl * 128:(tl + 1) * 128],
                                wv_t[:, di].rearrange(
                                    "p a b -> p (a b)"),
                                start=(di == 0), stop=(di == DT - 1))
                        tt = ch * (SC // 128) + tl
                        nc.vector.tensor_copy(v_sb[:, tt, :], pv[:])

            # ---- FUSED PASS: q projection + RoPE + attention ----
            # Per (head-group, s-tile): project q for HG heads straight
            # into SBUF, then run their causal attention.  Projection
            # matmuls of iteration i+1 overlap attention of iteration i.
            es = ExitStack()
            with es:
                es.enter_context(wqp_es.pop_all())
                es.enter_context(xtp_es.pop_all())
                wqp = es.enter_context(tc.tile_pool(name="wqp", bufs=1))
                rpB = es.enter_context(tc.tile_pool(name="ropeB", bufs=2))
                qjp = es.enter_context(tc.tile_pool(name="qj", bufs=5))
                wtp = es.enter_context(tc.tile_pool(name="wt", bufs=4))
                rcpp = es.enter_context(tc.tile_pool(name="rcp", bufs=2))
                onp = es.enter_context(tc.tile_pool(name="on", bufs=2))
                psq = es.enter_context(
                    tc.tile_pool(name="psqB", bufs=2, space="PSUM"))
                pss = es.enter_context(
                    tc.tile_pool(name="pss", bufs=3, space="PSUM"))
                pso = es.enter_context(
                    tc.tile_pool(name="pso", bufs=2, space="PSUM"))
                psd = es.enter_context(
                    tc.tile_pool(name="psd", bufs=1, space="PSUM"))

                def qproj_half(wq_h, qj, j, half):
                    xts = xtsp.tile([128, DT, SC], F32R, tag="xts",
                                    name="xtsF%d" % id(qj))
                    s0 = j * ST + half * SC
                    nc.sync.dma_start(xts[:], xT_r[:, :, s0:s0 + SC])
                    for hl in range(HG):
                        pq = psq.tile([128, SC], F32, tag="pq")
                        for di in range(DT):
                            nc.tensor.matmul(
                                pq[:], wq_h[hl][:, di, :], xts[:, di, :],
                                start=(di == 0), stop=(di == DT - 1))
                        rope(pq[:], qj[hl][:, half * SC:(half + 1) * SC],
                             s0, SC, rpB, pss, "ps")

                def attn_head(h, qjt, j):
                    kh = h // G
                    po = pso.tile([128, ST], F32, tag="po")
                    pden = psd.tile([1, ST], F32, tag="pden")
                    KT = (j + 1) * TJ
                    for kt in range(KT):
                        ps = pss.tile([128, ST], F32, tag="ps")
                        nc.tensor.matmul(
                            ps[:], k_sb[:, kh, kt * 128:(kt + 1) * 128],
                            qjt[:], start=True, stop=True)
                        wtile = wtp.tile([128, ST], F32R, tag="wt")
                        m = kt - j * TJ
                        lo = 0
                        if m >= 0:
                            nc.vector.tensor_tensor(
                                ps[:, m * 128:(m + 1) * 128],
                                ps[:, m * 128:(m + 1) * 128],
                                tri_sb[:], ADD)
                            lo = m * 128
                            if m > 0:
                                nc.scalar.mul(wtile[:, 0:lo], ps[:, 0:lo], 0.0)
                        nc.scalar.activation(wtile[:, lo:ST], ps[:, lo:ST],
                                             EXP, scale=scale)
                        nc.tensor.matmul(
                            po[:], v_sb[:, kt, kh * H:(kh + 1) * H], wtile[:],
                            start=(kt == 0), stop=(kt == KT - 1))
                        nc.tensor.matmul(
                            pden[:], ones_col, wtile[:],
                            start=(kt == 0), stop=(kt == KT - 1))
                    rcp = rcpp.tile([1, ST], F32, tag="rcp")
                    nc.vector.reciprocal(rcp[:], pden[:])
                    nc.gpsimd.dma_start(den_d[h][:, j * ST:(j + 1) * ST],
                                        rcp[:])
                    on = onp.tile([128, ST], F32R, tag="on")
                    nc.vector.tensor_copy(on[:], po[:])
                    nc.gpsimd.dma_start(oda[h][:, j * ST:(j + 1) * ST],
                                        on[:])

                for hg in range(QH // HG):
                    wq_h = []
                    for hl in range(HG):
                        if hg == 0 and hl == 0:
                            wq_h.append(wq_pre)
                            continue
                        pool_ = wqp0 if hl == 0 else wqp
                        wt_ = pool_.tile([128, DT, H], F32R, tag="wq%d" % hl,
                                         name="wq_%d_%d" % (hg, hl))
                        nc.sync.dma_start(
                            wt_[:],
                            wq.ap()[hg * HG + hl].rearrange(
                                "(dt p) h -> p dt h", p=128))
                        wq_h.append(wt_)
                    for j in range(NJ):
                        qj = [qjp.tile([128, ST], F32R, tag="qj",
                                       name="qj%d_%d_%d" % (hg, j, hl))
                              for hl in range(HG)]
                        for half in range(ST // SC):
                            qproj_half(wq_h, qj, j, half)
                        for hl in range(HG):
                            attn_head(hg * HG + hl, qj[hl], j)

            # ---- PASS C: o projection (partial over this core's heads) ----
            with nc.named_scope("passC"), \
                 tc.tile_pool(name="otp", bufs=1) as otp, \
                 tc.tile_pool(name="wop", bufs=2) as wop, \
                 tc.tile_pool(name="ocp", bufs=3) as ocp, \
                 tc.tile_pool(name="psc", bufs=4, space="PSUM") as psc:
                ot = otp.tile([128, QH, S], F32R)
                for h in range(QH):
                    nc.sync.dma_start(ot[:, h, :], oda[h][:, :])
                with tc.tile_pool(name="bch", bufs=2) as bchp:
                    for h in range(QH):
                        bch = bchp.tile([128, S], F32, tag="bch",
                                        name="bch%d" % h)
                        nc.sync.dma_start(bch[:],
                                          den_d[h][:].partition_broadcast(128))
                        nc.vector.tensor_tensor(ot[:, h, :], ot[:, h, :],
                                                bch[:], MULT)
                with tc.tile_pool(name="bch", bufs=2) as bchp:
                    for h in range(QH):
                        bch = bchp.tile([128, S], F32, tag="bch",
                                        name="bch%d" % h)
                        nc.sync.dma_start(bch[:],
                                          den_d[h][:].partition_broadcast(128))
                        nc.vector.tensor_tensor(ot[:, h, :], ot[:, h, :],
                                                bch[:], MULT)
                for e in range(NE):
                    woe = wop.tile([128, QH, EW], F32R, tag="woe")
                    for h in range(QH):
                        nc.sync.dma_start(
                            woe[:, h, :], wo.ap()[h, :, e * EW:(e + 1) * EW])
                    for st in range(S // 128):
                        pc = psc.tile([128, EW], F32, tag="pc")
                        for h in range(QH):
                            nc.tensor.matmul(
                                pc[:],
                                ot[:, h, st * 128:(st + 1) * 128],
                                woe[:, h, :],
                                start=(h == 0), stop=(h == QH - 1))
                        oc = ocp.tile([128, EW], F32, tag="oc")
                        nc.vector.tensor_copy(oc[:], pc[:])
                        nc.scalar.dma_start(
                            o_out[st * 128:(st + 1) * 128, e * EW:(e + 1) * EW],
                            oc[:])

    nc.compile()
    return nc


def _perm_matrix():
    P = np.zeros((128, 128), dtype=np.float32)
    P[np.arange(128), (np.arange(128) + 64) % 128] = 1.0
    return P


def make_tables(positions_b, S, H):
    """cos/sin tables in [128, S] layout with the sign fold for the swap
    trick (rows 0:63 -> +sin, 64:127 -> -sin), plus the triangular mask."""
    half = H // 2
    inv_freq = 1.0 / (ROPE_THETA ** (np.arange(half, dtype=np.float64) * 2.0 / H))
    ang = positions_b.astype(np.float64)[None, :] * inv_freq[:, None]  # [half, S]
    cos_h = np.cos(ang)
    sin_h = np.sin(ang)
    cos_t = np.concatenate([cos_h, cos_h], axis=0).astype(np.float32)
    sin_t = np.concatenate([sin_h, -sin_h], axis=0).astype(np.float32)
    idx = np.arange(128)
    tri = np.where(idx[:, None] <= idx[None, :], 0.0, NEG_BIG).astype(np.float32)
    return cos_t, sin_t, tri


def make_in_maps(x, positions, Wq, Wk, Wv, Wo, cfg):
    """Shard the full inputs into the 8 per-core input maps."""
    QH, KH = cfg["QH"], cfg["KH"]
    S, H = cfg["S"], cfg["H"]
    B = x.shape[0]
    groups = N_CORES // B
    tables = [make_tables(np.asarray(positions[b]), S, H) for b in range(B)]
    in_maps = []
    for c in range(N_CORES):
        b, g = divmod(c, groups)
        cos_t, sin_t, tri = tables[b]
        in_maps.append({
            "xT": np.ascontiguousarray(np.asarray(x[b]).T),
            "wq": np.ascontiguousarray(Wq[g * QH:(g + 1) * QH]),
            "wk": np.ascontiguousarray(Wk[g * KH:(g + 1) * KH]),
            "wv": np.ascontiguousarray(Wv[g * KH:(g + 1) * KH]),
            "wo": np.ascontiguousarray(Wo[g * QH:(g + 1) * QH]),
            "cos_t": cos_t,
            "sin_t": sin_t,
            "tri_t": tri,
            "ones_t": np.ones((128, 128), dtype=np.float32),
            "perm_t": _perm_matrix(),
        })
    return in_maps


_NC_CACHE = {}


def _get_nc(cfg_key=None):
    cfg = FULL_CFG if cfg_key is None else cfg_key
    key = tuple(sorted(cfg.items()))
    if key not in _NC_CACHE:
        _NC_CACHE[key] = build_bass(cfg)
    return _NC_CACHE[key]


def run(x, positions, Wq, Wk, Wv, Wo, trace=False, trace_kwargs=None):
    cfg = FULL_CFG
    nc = _get_nc(cfg)
    in_maps = make_in_maps(np.asarray(x), np.asarray(positions),
                           np.asarray(Wq), np.asarray(Wk), np.asarray(Wv),
                           np.asarray(Wo), cfg)
    res = bass_utils.run_bass_kernel_spmd(
        nc, in_maps, list(range(N_CORES)), trace=trace,
        **(trace_kwargs or {}))
    B = np.asarray(x).shape[0]
    groups = N_CORES // B
    outs = []
    for b in range(B):
        acc = res.results[b * groups]["o_out"].astype(np.float64)
        for g in range(1, groups):
            acc += res.results[b * groups + g]["o_out"]
        outs.append(acc.astype(np.float32))
    return np.stack(outs, axis=0), res


def kernel(x, positions, Wq, Wk, Wv, Wo):
    out, _ = run(x, positions, Wq, Wk, Wv, Wo, trace=False)
    return out
